# revision 41
# baseline (speedup 1.0000x reference)
"""Multi-head attention (B=2, S=2048, D=1024, H=16, hd=64, RoPE, causal)
on 8 Trainium2 NeuronCores.

Sharding: each core owns 2 heads x both batches (tensor-parallel over heads).
Out-projection is sharded by sequence eighths (each core owns 256 positions
of each batch), fed by one AllToAll per batch so the first collective hides
behind batch-1 compute.

Engine plan per core:
  PE    : QKV projections, scores (transposed), attn@V, out-proj.  The
          attention inner loop is software-pipelined (scores kt | attnV kt-1,
          heads interleaved) and padded with "filler" matmuls (next batch's
          projections / out-proj) so the PE stays continuously busy and
          p-state-ramps to full clock.
  ACT   : exp() only.
  DVE   : PSUM->SBUF copies (+bias), rope multiplies (all-bf16 SBUF for the
          4x DVE mode), causal-mask adds, softmax reciprocal + normalize.
  Pool  : bulk weight/x DMAs (software DGE) + collectives + post-collective
          loads, so the sync/scalar queues stay shallow.
  DMA   : rope half-swap as SBUF->SBUF partition-moving copies.
"""
import itertools
import os

import ml_dtypes
import numpy as np

import concourse.bass as bass
import concourse.mybir as mybir
import concourse.tile as tile
from concourse.bass_utils import run_bass_kernel_spmd
from concourse.vector_clock import ScopedClock

B, S, D, H, HD = 2, 2048, 1024, 16, 64
NCORES = 8
HPC = 2                    # heads per core
F = HPC * HD               # 128 features per core
CHUNK = 512
QTR = 256                  # out-proj positions per core per batch
NCH = S // CHUNK           # 4 q-chunks
NKT = D // 128             # 8 contraction tiles for projections
NST = S // 128             # 16 key tiles
F32 = mybir.dt.float32
F32R = mybir.dt.float32r
BF16 = mybir.dt.bfloat16
NPBF16 = ml_dtypes.bfloat16


# ---------------------------------------------------------------------------
# Workarounds for the walrus build in this container: it encodes at most ONE
# sync-wait per instruction ("Too many sync wait commands"). Split multi-wait
# instructions into single-wait NoOps. Semantics-preserving.
# ---------------------------------------------------------------------------
_patched = False


def _install_patches():
    global _patched
    if _patched:
        return
    _patched = True

    _orig_lower = tile.TileContext._lower_ordered_insts

    def _lower_with_wait_split(self, ordered):
        nc = self.nc
        for _bb, insts in ordered.items():
            if not any(
                i.sync_info is not None and len(i.sync_info.on_wait) > 1
                for i in insts
            ):
                continue
            new = []
            for inst in insts:
                si = inst.sync_info
                if si is not None and len(si.on_wait) > 1:
                    waits = list(si.on_wait)
                    for w in waits[:-1]:
                        n = mybir.InstNoOp(
                            name=f"I-waitsplit-{nc.next_id()}", ins=[], outs=[]
                        )
                        n.engine = inst.engine
                        n.bass_nofuse = True
                        n.sync_info = mybir.SyncInfo(on_wait=[w], on_update=[])
                        nc.register_instruction(n)
                        new.append(n)
                    inst.sync_info = mybir.SyncInfo(
                        on_wait=[waits[-1]], on_update=list(si.on_update)
                    )
                new.append(inst)
            insts[:] = new
        return _orig_lower(self, ordered)

    tile.TileContext._lower_ordered_insts = _lower_with_wait_split

    def _drain_and_barrier(self, tick_clock, wait_clock):
        nc = self.nc
        probe = nc.sync.nop(nofuse=True)
        wait_clock.add_sem_waits(
            probe.ins, ScopedClock({None: tick_clock.global_clock})
        )
        waits = list(probe.ins.sync_info.on_wait)
        probe.ins.sync_info = mybir.SyncInfo(on_wait=waits[:1], on_update=[])
        for w in waits[1:]:
            n2 = nc.sync.nop(nofuse=True)
            n2.ins.sync_info = mybir.SyncInfo(on_wait=[w], on_update=[])
        nc.sync.drain()
        nc.all_engine_barrier()
        assert self.sems is not None
        popped = nc._tile_sem_poison_stack.pop()
        assert popped is self._sem_poison
        nc.clear_and_free_semaphores(list(self.sems.allocated().values()))
        nc.all_engine_barrier()

    tile.TileContext._drain_and_barrier = _drain_and_barrier


def _install_ntff_hook():
    """Provide the missing ``antenv.axon_hooks`` module so trace=True works."""
    import sys
    import types

    if "antenv.axon_hooks" in sys.modules:
        return
    try:
        import antenv
        from trn_agent_boot.trn_boot import _ntff_profile_via_ctypes
    except ImportError:
        return
    mod = types.ModuleType("antenv.axon_hooks")
    mod._hook = _ntff_profile_via_ctypes("/opt/axon/libaxon_pjrt.so")
    mod.set_axon_ntff_profile_hook = lambda h: setattr(mod, "_hook", h)
    mod.get_axon_ntff_profile_hook = lambda: mod._hook
    sys.modules["antenv.axon_hooks"] = mod
    antenv.axon_hooks = mod


def _bcast_parts(ap, nparts):
    """Partition-broadcast view of a single-partition AP (stride-0)."""
    return bass.AP(
        tensor=ap.tensor, offset=ap.offset,
        ap=[[0, nparts]] + [list(p) for p in ap.ap[1:]],
    )


# ---------------------------------------------------------------------------
# Program builder (same program on all 8 cores; per-core data differs)
# ---------------------------------------------------------------------------
def build_program():
    _install_patches()
    nc = bass.Bass(num_devices=NCORES)

    xt = [nc.dram_tensor(f"xt{b}", [D, S], BF16, kind="ExternalInput")
          for b in range(B)]
    wqkvt = nc.dram_tensor("wqkvt", [D, 3 * F], BF16, kind="ExternalInput")
    bq = nc.dram_tensor("bq", [F], F32, kind="ExternalInput")
    bk = nc.dram_tensor("bk", [F], F32, kind="ExternalInput")
    bv = nc.dram_tensor("bv", [F], F32, kind="ExternalInput")
    wot = nc.dram_tensor("wot", [D, D], BF16, kind="ExternalInput")
    bo = nc.dram_tensor("bo", [D], F32, kind="ExternalInput")
    chat = nc.dram_tensor("chat", [F, S], BF16, kind="ExternalInput")
    shat = nc.dram_tensor("shat", [F, S], BF16, kind="ExternalInput")
    ident_in = nc.dram_tensor("ident128", [128, 128], BF16, kind="ExternalInput")
    perm_in = nc.dram_tensor("perm128", [128, 128], BF16, kind="ExternalInput")
    vones_in = nc.dram_tensor("vones", [NST, HPC], BF16, kind="ExternalInput")
    mask_in = nc.dram_tensor("mask128", [128, 128], F32, kind="ExternalInput")
    ytq = [nc.dram_tensor(f"ytq{b}", [D, QTR], F32, kind="ExternalOutput")
           for b in range(B)]

    ones_in = nc.dram_tensor("ones64", [1, 64], BF16, kind="ExternalInput")
    a2a_in = [nc.dram_tensor(f"a2a_in{b}", [NCORES, F * QTR], BF16)
              for b in range(B)]
    a2a_out = [nc.dram_tensor(f"a2a_out{b}", [NCORES, F * QTR], BF16)
               for b in range(B)]
    a2a_in3 = [t.rearrange("g (p n) -> g p n", p=F) for t in a2a_in]
    a2a_out3 = [t.rearrange("g (p n) -> g p n", p=F) for t in a2a_out]

    with tile.TileContext(nc) as tc:
        with (
            tc.tile_pool(name="const", bufs=1) as const,
            tc.tile_pool(name="wpool", bufs=1) as wpool,
            tc.tile_pool(name="xtp", bufs=1) as xtp,
            tc.tile_pool(name="qkv", bufs=2) as qkv,
            tc.tile_pool(name="vagg", bufs=2) as vaggp,
            tc.tile_pool(name="rope", bufs=2) as ropep,
            tc.tile_pool(name="expp", bufs=2) as expp,
            tc.tile_pool(name="normp", bufs=2) as normp,
            tc.tile_pool(name="stage", bufs=4) as stage,
            tc.tile_pool(name="at2", bufs=1) as at2p,
            tc.tile_pool(name="ys", bufs=2) as ysp,
            tc.tile_pool(name="ps", bufs=1, space="PSUM") as ps,
        ):
            # ---- constants + weights ----
            # pool queue (fast software DGE) carries the bulk stream in
            # need-order; sync/scalar queues carry the small tables.
            xs3 = [
                xt[b].rearrange("(k p) s -> p k s", p=128) for b in range(B)
            ]
            # batch-0 x as per-chunk tiles so chunk 0 lands first
            x0c = [xtp.tile([128, NKT * CHUNK], BF16, tag=f"xt0c{c}",
                            name=f"xt0c{c}") for c in range(NCH)]
            x0c3 = [t.rearrange("p (k s) -> p k s", s=CHUNK) for t in x0c]
            x1c = [xtp.tile([128, NKT * CHUNK], BF16, tag=f"xt1c{c}",
                            name=f"xt1c{c}") for c in range(NCH)]
            x1c3 = [t.rearrange("p (k s) -> p k s", s=CHUNK) for t in x1c]

            # sync queue: tiny tables first, then half of x(b0) chunk 0
            ident = const.tile([128, 128], BF16)
            nc.sync.dma_start(out=ident, in_=ident_in[:])
            perm = const.tile([128, 128], BF16)
            nc.sync.dma_start(out=perm, in_=perm_in[:])
            mask = const.tile([128, 128], F32)
            nc.sync.dma_start(out=mask, in_=mask_in[:])
            bq_t = const.tile([F, 1], F32)
            nc.sync.dma_start(out=bq_t, in_=bq.rearrange("(p o) -> p o", o=1))
            bk_t = const.tile([F, 1], F32)
            nc.sync.dma_start(out=bk_t, in_=bk.rearrange("(p o) -> p o", o=1))
            bv_t = const.tile([F, 1], F32)
            nc.sync.dma_start(out=bv_t, in_=bv.rearrange("(p o) -> p o", o=1))
            ones_t = const.tile([1, 64], BF16)
            nc.sync.dma_start(out=ones_t, in_=ones_in[:])
            bo_t = const.tile([128, NKT], F32)
            nc.sync.dma_start(out=bo_t, in_=bo.rearrange("(e p) -> p e", p=128))
            # scalar queue: x(b0) chunk-0 first half, then rope tables
            # (ACT is idle until the first exp, so nothing is stolen)
            nc.scalar.dma_start(
                out=x0c3[0][:, 0:4, :], in_=xs3[0][:, 0:4, 0:CHUNK]
            )
            chat_t = const.tile([F, S], BF16)
            nc.scalar.dma_start(out=chat_t, in_=chat[:])
            shat_t = const.tile([F, S], BF16)
            nc.scalar.dma_start(out=shat_t, in_=shat[:])

            # pool queue (fast software DGE): the bulk stream in need-order
            wqkv_all = wpool.tile([128, NKT * 3 * F], BF16, name="wqkv")
            w3 = wqkv_all.rearrange("p (k c) -> p k c", c=3 * F)
            ws3 = wqkvt.rearrange("(k p) c -> p k c", p=128)
            nc.gpsimd.dma_start(out=w3, in_=ws3)
            nc.gpsimd.dma_start(
                out=x0c3[0][:, 4:8, :], in_=xs3[0][:, 4:8, 0:CHUNK]
            )
            for c in range(1, NCH):
                nc.gpsimd.dma_start(
                    out=x0c3[c],
                    in_=xs3[0][:, :, CHUNK * c: CHUNK * (c + 1)],
                )
            for c in range(NCH):
                nc.gpsimd.dma_start(
                    out=x1c3[c],
                    in_=xs3[1][:, :, CHUNK * c: CHUNK * (c + 1)],
                )
            wo_all = wpool.tile([128, NKT * D], BF16, name="wo")
            wo3 = wo_all.rearrange("p (k c) -> p k c", c=D)
            wos3 = wot.rearrange("(k p) c -> p k c", p=128)
            nc.gpsimd.dma_start(out=wo3, in_=wos3)

            def xsl(b, k, c):
                if b == 0:
                    return x0c3[c][:, k, :]
                return x1c3[c][:, k, :]

            def wslice(name, k):
                base = {"q": 0, "k": F, "v": 2 * F}[name]
                return wqkv_all[:, k * 3 * F + base: k * 3 * F + base + F]

            bias_t = {"q": bq_t, "k": bk_t, "v": bv_t}

            # -------------------------------------------------------------
            # Projection "filler" units for batch b: list of closures, each
            # emitting one small instruction group. Consumed between
            # attention pipeline steps (or run back-to-back for batch 0).
            # -------------------------------------------------------------
            def proj_units(b, tags=("pj",), hook=None):
                QT = qkv.tile([F, S], BF16, tag="QT")
                KT = qkv.tile([F, S], BF16, tag="KT")
                VT = qkv.tile([F, S], BF16, tag="VT")
                vagg = vaggp.tile([128, NST, HPC * 65], BF16, tag="vagg")
                units = []
                tagc = [0]

                def next_tag():
                    t = tags[tagc[0] % len(tags)]
                    tagc[0] += 1
                    return t

                # ones column of the V augmentation (denominator lane)
                def ones_col(vagg=vagg):
                    vi = vones_in[:]
                    bcast = bass.AP(
                        tensor=vi.tensor, offset=vi.offset,
                        ap=[[0, 128]] + [list(p) for p in vi.ap],
                    )
                    nc.sync.dma_start(
                        out=vagg.rearrange("p st (h u) -> p st h u", u=65)
                            [:, :, :, 64],
                        in_=bcast,
                    )
                units.append(ones_col)

                for c in range(NCH):
                    cs = slice(CHUNK * c, CHUNK * (c + 1))
                    for name in ("q", "k", "v"):
                        tg = next_tag()
                        pm = ps.tile([F, CHUNK], F32, tag=tg,
                                     bufs=3 if tg == "mm" else 1,
                                     name="pm_proj")
                        for k in range(NKT):
                            def mm(pm=pm, k=k, b=b, c=c, name=name):
                                nc.tensor.matmul(
                                    pm, wslice(name, k), xsl(b, k, c),
                                    start=(k == 0), stop=(k == NKT - 1),
                                    skip_group_check=True,
                                )
                            units.append(mm)
                        if name == "v":
                            def vcopy(pm=pm, VT=VT, cs=cs):
                                nc.vector.tensor_scalar_add(
                                    VT[:, cs], pm, bv_t[:]
                                )
                            units.append(vcopy)
                            if hook is not None:
                                units.append(
                                    lambda c=c, VT=VT: hook(c, VT)
                                )

                            # V transpose for this chunk's 4 s-tiles
                            for st in range(4 * c, 4 * c + 4):
                                def vtr(VT=VT, vagg=vagg, st=st):
                                    pt = ps.tile([128, 128], BF16, tag="pt",
                                                 name="pt_vtr")
                                    nc.tensor.transpose(
                                        pt, VT[:, 128 * st:128 * (st + 1)],
                                        ident[:],
                                    )
                                    nc.vector.tensor_scalar_add(
                                        vagg.rearrange(
                                            "p st (h u) -> p st h u", u=65)
                                            [:, st, :, 0:64],
                                        pt.rearrange(
                                            "p (h u) -> p h u", h=HPC),
                                        0.0,
                                    )
                                units.append(vtr)
                        else:
                            dst = QT if name == "q" else KT

                            def rope(pm=pm, dst=dst, cs=cs, name=name):
                                raw = ropep.tile([F, CHUNK], BF16, tag="raw")
                                nc.vector.tensor_scalar_add(
                                    raw, pm, bias_t[name][:]
                                )
                                # half-swap as a PE permutation matmul
                                psw = ps.tile([F, CHUNK], F32, tag="aux",
                                              name="psw")
                                nc.tensor.matmul(psw, perm, raw,
                                                 start=True, stop=True,
                                                 skip_group_check=True)
                                t1 = ropep.tile([F, CHUNK], BF16, tag="t1")
                                nc.vector.tensor_mul(t1, raw, chat_t[:, cs])
                                t2 = ropep.tile([F, CHUNK], BF16, tag="t2")
                                nc.vector.tensor_mul(t2, psw, shat_t[:, cs])
                                nc.vector.tensor_add(dst[:, cs], t1, t2)
                            units.append(rope)
                return QT, KT, VT, vagg, units

            # -------------------------------------------------------------
            # Out-projection units for batch b (consumed as filler too)
            # -------------------------------------------------------------
            def outproj_units(b, at2, tags=("pj",)):
                units = []
                for et in range(NKT):
                    tg = tags[et % len(tags)]
                    pm = ps.tile([F, CHUNK], F32, tag=tg,
                                 bufs=3 if tg == "mm" else 1,
                                 name="pm_yproj")
                    for k in range(NKT):
                        def mm(pm=pm, k=k, et=et):
                            nc.tensor.matmul(
                                pm[:, 0:QTR],
                                wo_all[:, k * D + 128 * et: k * D + 128 * (et + 1)],
                                at2[k],
                                start=(k == 0), stop=(k == NKT - 1),
                                skip_group_check=True,
                            )
                        units.append(mm)

                    def fin(pm=pm, et=et, b=b):
                        ys = ysp.tile([128, QTR], F32, tag="ys")
                        nc.vector.tensor_scalar_add(
                            ys, pm[:, 0:QTR], bo_t[:, et:et + 1]
                        )
                        nc.sync.dma_start(
                            out=ytq[b][128 * et:128 * (et + 1), :], in_=ys
                        )
                    units.append(fin)
                return units

            # -------------------------------------------------------------
            # Attention for batch b with PE-filler interleaving.
            # filler: iterator of unit closures; rate: units per kt step.
            # -------------------------------------------------------------
            def attention(b, QT, KT, VT, vagg, fillers, rates, drain=True):
                fill_budget = 0.0
                filler = None

                def advance(n):
                    nonlocal fill_budget
                    fill_budget += n
                    while fill_budget >= 1.0:
                        u = next(filler, None)
                        if u is None:
                            fill_budget = 0.0
                            return
                        u()
                        fill_budget -= 1.0

                for c in range(NCH):
                    filler = fillers[c]
                    nkt = 4 * c + 4
                    ex_t = {}
                    av = [
                        ps.tile([65, CHUNK], F32, tag=f"av{h}", name=f"av{h}")
                        for h in range(HPC)
                    ]
                    for kt in range(nkt):
                        qlo = max(CHUNK * c, 128 * kt)
                        w = CHUNK * (c + 1) - qlo
                        for h in range(HPC):
                            hs = slice(64 * h, 64 * (h + 1))
                            pm = ps.tile([128, CHUNK], F32, tag="mm",
                                         bufs=3, name=f"pm_s{h}")
                            nc.tensor.matmul(
                                pm[:, 0:w],
                                KT[hs, 128 * kt:128 * (kt + 1)],
                                QT[hs, qlo:qlo + w],
                                start=True, stop=True,
                            )
                            if 128 * kt >= CHUNK * c:
                                nc.vector.tensor_add(
                                    pm[:, 0:128], pm[:, 0:128], mask[:]
                                )
                            ex = expp.tile([128, CHUNK], BF16, tag=f"exp{h}")
                            nc.scalar.activation(
                                ex[:, 0:w], pm[:, 0:w],
                                mybir.ActivationFunctionType.Exp,
                                scale=0.125,
                            )
                            ex_t[(h, kt)] = ex

                        def attnv(kt):
                            qlo2 = max(CHUNK * c, 128 * kt)
                            w2 = CHUNK * (c + 1) - qlo2
                            off = qlo2 - CHUNK * c
                            for h in range(HPC):
                                nc.tensor.matmul(
                                    av[h][:, off:CHUNK],
                                    vagg[:, kt, 65 * h:65 * (h + 1)],
                                    ex_t.pop((h, kt))[:, 0:w2],
                                    start=(kt == 0), stop=(kt == nkt - 1),
                                    skip_group_check=True,
                                )

                        if kt > 0:
                            attnv(kt - 1)
                        advance(rates[c])
                    attnv(nkt - 1)

                    # normalize + stage this chunk for the all-to-all:
                    # 1/denominator on DVE, broadcast across 64 partitions
                    # via a tiny PE matmul, multiply out of PSUM.
                    for h in range(HPC):
                        hs = slice(64 * h, 64 * (h + 1))
                        lnb = normp.tile([1, CHUNK], F32, tag="lnb")
                        nc.scalar.activation(
                            lnb, av[h][64:65, :],
                            mybir.ActivationFunctionType.Ln,
                        )
                        recbr = normp.tile([1, CHUNK], BF16, tag="recbr")
                        nc.scalar.activation(
                            recbr, lnb,
                            mybir.ActivationFunctionType.Exp, scale=-1.0,
                        )
                        aux = ps.tile([F, CHUNK], F32, tag="aux",
                                      name="pb")
                        pb = aux[0:64, :]
                        nc.tensor.matmul(
                            pb, ones_t, recbr,
                            start=True, stop=True, skip_group_check=True,
                        )
                        pbs = normp.tile([64, CHUNK], F32, tag="pbs")
                        nc.vector.tensor_scalar_add(pbs, pb, 0.0)
                        sg = stage.tile([64, CHUNK], BF16, tag="sg")
                        nc.vector.tensor_mul(sg, av[h][0:64, :], pbs)
                        for half in range(2):
                            nc.sync.dma_start(
                                out=a2a_in3[b][2 * c + half][hs, :],
                                in_=sg[:, QTR * half: QTR * (half + 1)],
                            )
                if drain:
                    advance(10 ** 6)

            # ================= schedule =================
            QT0, KT0, VT0, vagg0, pu0 = proj_units(0, tags=("mm",))
            for u in pu0:
                u()

            QT1, KT1, VT1, vagg1, pu1 = proj_units(1, tags=("pj",))
            # batch-0 attention, feeding batch-1 projections into PE gaps;
            # leftovers spill into batch-1 attention's early chunks (before
            # the first collective's output is usable as filler).
            pu1_it = iter(pu1)
            attention(0, QT0, KT0, VT0, vagg0, [pu1_it] * 4,
                      rates=(2.0, 2.6, 3.4, 3.4), drain=False)

            # first collective: batch-0 heads -> sequence eighths
            nc.gpsimd.collective_compute(
                "AllToAll",
                mybir.AluOpType.bypass,
                replica_groups=[list(range(NCORES))],
                ins=[a2a_in[0][:]],
                outs=[a2a_out[0][:]],
            )
            at2a_0 = at2p.tile([128, NCORES * QTR], BF16, tag="at0",
                               name="at0")
            nc.gpsimd.dma_start(
                out=at2a_0.rearrange("p (g n) -> p g n", n=QTR),
                in_=a2a_out[0].rearrange("g (p n) -> p g n", p=F),
            )
            at2_0 = [at2a_0[:, QTR * g: QTR * (g + 1)]
                     for g in range(NCORES)]

            # batch-1 attention: early chunks consume leftover projection
            # units; from c>=2 the batch-0 out-projection (collective #1 has
            # landed by then) fills PE gaps.
            op0_it = iter(outproj_units(0, at2_0))
            attention(1, QT1, KT1, VT1, vagg1,
                      [pu1_it, pu1_it,
                       itertools.chain(pu1_it, op0_it),
                       itertools.chain(pu1_it, op0_it)],
                      rates=(2.4, 2.4, 0.0, 2.5), drain=False)

            nc.gpsimd.collective_compute(
                "AllToAll",
                mybir.AluOpType.bypass,
                replica_groups=[list(range(NCORES))],
                ins=[a2a_in[1][:]],
                outs=[a2a_out[1][:]],
            )
            # finish the held-back batch-0 out-projection while the second
            # collective is in flight (keeps the PE busy and p-state-ramped)
            for u in op0_it:
                u()
            at2a_1 = at2p.tile([128, NCORES * QTR], BF16, tag="at1",
                               name="at1")
            nc.gpsimd.dma_start(
                out=at2a_1.rearrange("p (g n) -> p g n", n=QTR),
                in_=a2a_out[1].rearrange("g (p n) -> p g n", p=F),
            )
            at2_1 = [at2a_1[:, QTR * g: QTR * (g + 1)]
                     for g in range(NCORES)]

            for u in outproj_units(1, at2_1, tags=("mm",)):
                u()

    nc.finalize()
    return nc


_NC_CACHE = None


def _get_program():
    global _NC_CACHE
    if _NC_CACHE is None:
        _NC_CACHE = build_program()
    return _NC_CACHE


def _prep_in_maps(x, cos, sin, Wq, bq, Wk, bk, Wv, bv, Wo, bo):
    cosT = np.ascontiguousarray(cos.T).astype(np.float32)    # (32, S)
    sinT = np.ascontiguousarray(sin.T).astype(np.float32)
    chat = np.concatenate([cosT, cosT, cosT, cosT], 0).astype(NPBF16)
    shat = np.concatenate([-sinT, sinT, -sinT, sinT], 0).astype(NPBF16)
    xT = [np.ascontiguousarray(x[b].T).astype(NPBF16) for b in range(B)]
    mask128 = np.where(np.arange(128)[:, None] > np.arange(128)[None, :],
                       np.float32(-240.0), np.float32(0.0)).astype(np.float32)
    wqT, wkT, wvT = (np.ascontiguousarray(W.T).astype(NPBF16)
                     for W in (Wq, Wk, Wv))
    sw = np.arange(128); sw = np.where((sw // 32) % 2 == 0, sw + 32, sw - 32)
    perm128 = np.zeros((128, 128), np.float32)
    perm128[sw, np.arange(128)] = 1.0
    perm128 = perm128.astype(NPBF16)
    woT = np.ascontiguousarray(Wo.T).astype(NPBF16)

    in_maps = []
    for core in range(NCORES):
        sl = slice(F * core, F * (core + 1))
        wqkv = np.empty((D, 3 * F), NPBF16)
        wqkv[:, 0:F] = wqT[:, sl]
        wqkv[:, F:2 * F] = wkT[:, sl]
        wqkv[:, 2 * F:3 * F] = wvT[:, sl]
        in_maps.append({
            "xt0": xT[0], "xt1": xT[1],
            "wqkvt": wqkv,
            "bq": np.ascontiguousarray(bq[sl]),
            "bk": np.ascontiguousarray(bk[sl]),
            "bv": np.ascontiguousarray(bv[sl]),
            "wot": woT, "bo": bo,
            "chat": chat, "shat": shat,
            "ident128": np.eye(128, dtype=np.float32).astype(NPBF16),
            "perm128": perm128,
            "vones": np.ones((NST, HPC), NPBF16),
            "mask128": mask128,
            "ones64": np.ones((1, 64), NPBF16),
        })
    return in_maps


def kernel(x, cos, sin, mask, Wq, bq, Wk, bk, Wv, bv, Wo, bo, **_unused):
    """Full inputs in, full output out. `mask` (the causal mask) is
    regenerated on-device, so the input tensor itself is unused."""
    x, cos, sin = (np.asarray(a, np.float32) for a in (x, cos, sin))
    Wq, bq, Wk, bk = (np.asarray(a, np.float32) for a in (Wq, bq, Wk, bk))
    Wv, bv, Wo, bo = (np.asarray(a, np.float32) for a in (Wv, bv, Wo, bo))

    nc = _get_program()
    in_maps = _prep_in_maps(x, cos, sin, Wq, bq, Wk, bk, Wv, bv, Wo, bo)

    trace = bool(int(os.environ.get("MHA_TRACE", "0")))
    kw = {}
    if trace:
        _install_ntff_hook()
        kw = dict(trace=True, trace_cores=list(range(NCORES)))
    res = run_bass_kernel_spmd(nc, in_maps, core_ids=list(range(NCORES)), **kw)
    kernel.last_results = res

    y = np.empty((B, S, D), np.float32)
    for r in range(NCORES):
        for b in range(B):
            y[b, QTR * r:QTR * (r + 1), :] = res.results[r][f"ytq{b}"].T
    return y
